# revision 20
# baseline (speedup 1.0000x reference)
"""BetweennessRoPE Trainium2 kernel — fixed-table interpolated RoPE (v3).

Best measured: 38,085 ns, rel err 1.079e-3.  See kernel.py header for
the math derivation; this variant uses 8 uniform 2048-col stages.
"""

import numpy as np

B, S, H, D = 4, 2048, 16, 128
NCORES = 8
NSB = 2
K2 = D // 2
NJ = B * H
UW = 2048
NU = 8
NJU = H
PE_UNITS = (0, 1, 2, 3, 4, 5)
FR = 0.95 + 0.5 / 2046.0 * 0.1

_cache = {}


def _make_tables():
    k = np.arange(K2, dtype=np.float64)
    base = 1.0 / (10000.0 ** (2.0 * k / D))
    ang = np.arange(S, dtype=np.float64)[:, None] * base[None, :]
    fcos, fsin = np.cos(ang), np.sin(ang)
    lo = np.maximum(np.arange(S) - 1, 0)
    C = (1.0 - FR) * fcos[lo] + FR * fcos
    Sn = (1.0 - FR) * fsin[lo] + FR * fsin
    C[0, :] = 1.0
    Sn[0, :] = 0.0
    CD = np.repeat(C, 2, axis=1)
    SD = np.empty((S, D), np.float64)
    SD[:, 0::2] = -Sn
    SD[:, 1::2] = Sn
    return CD.astype(np.float16), SD.astype(np.float16)


def _build_nc():
    import concourse.bacc as bacc
    import concourse.mybir as mybir
    from concourse.tile import TileContext

    f16 = mybir.dt.float16
    f32 = mybir.dt.float32

    nc = bacc.Bacc()
    X = nc.dram_tensor("X", [NU, 128, UW], f16, kind="ExternalInput")
    TW = NSB * 2 * D + 128
    TAB = nc.dram_tensor("TAB", [128, TW], f16, kind="ExternalInput")
    OUT = nc.dram_tensor("OUT", [NU, 128, UW], f16, kind="ExternalOutput")

    with TileContext(nc) as tc:
        with (
            tc.tile_pool(name="tab", bufs=1) as tabp,
            tc.tile_pool(name="xin", bufs=1) as xinp,
            tc.tile_pool(name="prod", bufs=1) as prodp,
            tc.tile_pool(name="out", bufs=4) as outp,
            tc.tile_pool(name="odve", bufs=2) as odvep,
            tc.tile_pool(name="ps", bufs=2, space="PSUM") as psp,
        ):
            tab = tabp.tile([128, TW], f16, tag="tab", name="tab")
            idt = tab[:, NSB * 2 * D:NSB * 2 * D + 128]

            xts = []
            for u in range(NU):
                x = xinp.tile([128, UW], f16, tag=f"x{u}", name=f"x{u}")
                if u == 0:
                    # two half loads so the first multiply starts after
                    # 256 KiB instead of 512 KiB; TAB rides in between
                    x0a = xinp.tile([128, 1024], f16, tag="x0a", name="x0a")
                    nc.sync.dma_start(x0a[:, :], X[0][:, 0:1024])
                    nc.sync.dma_start(tab[:, :], TAB[:, :])
                    nc.sync.dma_start(x[:, 0:1024], X[0][:, 1024:2048])
                else:
                    nc.sync.dma_start(x[:, :], X[u])
                xts.append(x)

            def stage_muls(xsrc, cb, sdb, m1d, m2d, nj):
                xv = xsrc.rearrange("p (j d) -> p j d", d=D)
                xsw = xsrc.rearrange(
                    "p (j k two) -> p j k two", two=2, k=K2)[:, :, :, ::-1]
                m1v = m1d.rearrange("p (j d) -> p j d", d=D)
                m2v = m2d.rearrange("p (j k two) -> p j k two", two=2, k=K2)
                nc.vector.tensor_mul(m1v, xv, cb)
                nc.vector.tensor_mul(m2v, xsw, sdb)

            m1s, m2s = [], []
            for u in range(NU):
                sb = u // 4
                x = xts[u]
                cb = tab[:, sb * 2 * D:sb * 2 * D + D].unsqueeze(
                    1).broadcast_to([128, NJU, D])
                sdb = (tab[:, sb * 2 * D + D:sb * 2 * D + 2 * D]
                       .rearrange("p (k two) -> p k two", two=2)
                       .unsqueeze(1).broadcast_to([128, NJU, K2, 2]))
                m1 = prodp.tile([128, UW], f16, tag=f"m1_{u}",
                                name=f"m1_{u}")
                m2 = prodp.tile([128, UW], f16, tag=f"m2_{u}",
                                name=f"m2_{u}")
                cbh = tab[:, sb * 2 * D:sb * 2 * D + D].unsqueeze(
                    1).broadcast_to([128, NJU // 2, D])
                sdh = (tab[:, sb * 2 * D + D:sb * 2 * D + 2 * D]
                       .rearrange("p (k two) -> p k two", two=2)
                       .unsqueeze(1).broadcast_to([128, NJU // 2, K2, 2]))
                if u == 0:
                    stage_muls(x0a[:, :], cbh, sdh,
                               m1[:, 0:1024], m2[:, 0:1024], NJU // 2)
                    stage_muls(x[:, 0:1024], cbh, sdh,
                               m1[:, 1024:2048], m2[:, 1024:2048], NJU // 2)
                else:
                    stage_muls(x[:, :], cb, sdb, m1[:, :], m2[:, :], NJU)
                m1s.append(m1)
                m2s.append(m2)
                if u == NU - 2:
                    # combine unit 6 between the stage-6 and stage-7
                    # multiplies so its output drains under the final
                    # muls; its DMA is issued after the PE units' (the
                    # in-order sync DGE must dispatch by readiness)
                    o6 = odvep.tile([128, UW], f16, tag="od", name="o6")
                    nc.vector.tensor_add(o6[:, :], m1[:, :], m2[:, :])

            for u in range(NU):
                if u == NU - 2:
                    nc.sync.dma_start(OUT[u], o6[:, :])
                    continue
                o = outp.tile([128, UW], f16, tag="o", name=f"o{u}")
                if u in PE_UNITS:
                    ps = psp.tile([128, UW], f32, tag="ps", name=f"ps{u}")
                    for q in range(UW // 512):
                        qs = slice(512 * q, 512 * (q + 1))
                        nc.tensor.matmul(ps[:, qs], idt, m1s[u][:, qs],
                                         start=True, stop=False)
                        nc.tensor.matmul(ps[:, qs], idt, m2s[u][:, qs],
                                         start=False, stop=True)
                    nc.scalar.copy(o[:, :], ps[:, :])
                    nc.sync.dma_start(OUT[u], o[:, :])
                else:
                    # final unit in shrinking pieces for a short drain
                    for a, z in ((0, 1024), (1024, 1536), (1536, 2048)):
                        nc.vector.tensor_add(o[:, a:z], m1s[u][:, a:z],
                                             m2s[u][:, a:z])
                        nc.sync.dma_start(OUT[u][:, a:z], o[:, a:z])
    nc.compile()
    return nc


def _get_built():
    if "nc" not in _cache:
        _cache["nc"] = _build_nc()
    return _cache["nc"]


def kernel(x, W, b):
    from concourse.bass_utils import run_bass_kernel_spmd

    assert x.shape == (B, S, H, D)
    x6 = np.asarray(x, dtype=np.float32).reshape(
        B, NCORES, NSB, 128, H, D).astype(np.float16)
    xs = np.ascontiguousarray(x6.transpose(1, 2, 0, 3, 4, 5)).reshape(
        NCORES, NU, 128, UW)

    if "tabs" not in _cache:
        CDf, SDf = _make_tables()
        cc = CDf.reshape(NCORES, NSB, 128, D)
        ss = SDf.reshape(NCORES, NSB, 128, D)
        tabs = np.empty((NCORES, 128, NSB * 2 * D + 128), np.float16)
        for sb in range(NSB):
            tabs[:, :, sb * 2 * D:sb * 2 * D + D] = cc[:, sb]
            tabs[:, :, sb * 2 * D + D:sb * 2 * D + 2 * D] = ss[:, sb]
        tabs[:, :, NSB * 2 * D:] = np.eye(128, dtype=np.float16)[None]
        _cache["tabs"] = np.ascontiguousarray(tabs)
    tabs = _cache["tabs"]

    nc = _get_built()
    in_maps = []
    for c in range(NCORES):
        in_maps.append({"X": xs[c], "TAB": tabs[c]})
    res = run_bass_kernel_spmd(nc, in_maps, core_ids=list(range(NCORES)))
    if res.exec_time_ns is not None:
        print(f"HW exec time: {res.exec_time_ns} ns")

    outs = np.stack([res.results[c]["OUT"] for c in range(NCORES)])
    full = outs.reshape(NCORES, NSB, B, 128, H, D).transpose(2, 0, 1, 3, 4, 5)
    return np.ascontiguousarray(full.reshape(B, S, H, D).astype(np.float32))


# revision 21
# speedup vs baseline: 1.1521x; 1.1521x over previous
"""BetweennessRoPE Trainium2 kernel — fixed-table interpolated RoPE (v3).

Best measured: 38,085 ns, rel err 1.079e-3.  See kernel.py header for
the math derivation; this variant uses 8 uniform 2048-col stages.
"""

import numpy as np

B, S, H, D = 4, 2048, 16, 128
NCORES = 8
NSB = 2
K2 = D // 2
NJ = B * H
UW = 2048
NU = 8
NJU = H
PE_UNITS = (0, 1, 2, 3, 4, 5)
FR = 0.95 + 0.5 / 2046.0 * 0.1

_cache = {}


def _make_tables():
    k = np.arange(K2, dtype=np.float64)
    base = 1.0 / (10000.0 ** (2.0 * k / D))
    ang = np.arange(S, dtype=np.float64)[:, None] * base[None, :]
    fcos, fsin = np.cos(ang), np.sin(ang)
    lo = np.maximum(np.arange(S) - 1, 0)
    C = (1.0 - FR) * fcos[lo] + FR * fcos
    Sn = (1.0 - FR) * fsin[lo] + FR * fsin
    C[0, :] = 1.0
    Sn[0, :] = 0.0
    CD = np.repeat(C, 2, axis=1)
    SD = np.empty((S, D), np.float64)
    SD[:, 0::2] = -Sn
    SD[:, 1::2] = Sn
    return CD.astype(np.float16), SD.astype(np.float16)


def _build_nc():
    import concourse.bacc as bacc
    import concourse.mybir as mybir
    from concourse.tile import TileContext

    f16 = mybir.dt.float16
    f32 = mybir.dt.float32

    nc = bacc.Bacc()
    X = nc.dram_tensor("X", [NU, 128, UW], f16, kind="ExternalInput")
    TW = NSB * 2 * D + 128
    TAB = nc.dram_tensor("TAB", [128, TW], f16, kind="ExternalInput")
    OUT = nc.dram_tensor("OUT", [NU, 128, UW], f16, kind="ExternalOutput")

    with TileContext(nc) as tc:
        with (
            tc.tile_pool(name="tab", bufs=1) as tabp,
            tc.tile_pool(name="xin", bufs=1) as xinp,
            tc.tile_pool(name="prod", bufs=1) as prodp,
            tc.tile_pool(name="out", bufs=4) as outp,
            tc.tile_pool(name="odve", bufs=2) as odvep,
            tc.tile_pool(name="ps", bufs=2, space="PSUM") as psp,
        ):
            tab = tabp.tile([128, TW], f16, tag="tab", name="tab")
            idt = tab[:, NSB * 2 * D:NSB * 2 * D + 128]

            xts = []
            for u in range(NU):
                x = xinp.tile([128, UW], f16, tag=f"x{u}", name=f"x{u}")
                nc.sync.dma_start(x[:, :], X[u])
                xts.append(x)
                if u == 0:
                    nc.sync.dma_start(tab[:, :], TAB[:, :])

            m1s, m2s = [], []
            for u in range(NU):
                sb = u // 4
                x = xts[u]
                cb = tab[:, sb * 2 * D:sb * 2 * D + D].unsqueeze(
                    1).broadcast_to([128, NJU, D])
                sdb = (tab[:, sb * 2 * D + D:sb * 2 * D + 2 * D]
                       .rearrange("p (k two) -> p k two", two=2)
                       .unsqueeze(1).broadcast_to([128, NJU, K2, 2]))
                xv = x[:, :].rearrange("p (j d) -> p j d", d=D)
                xsw = x[:, :].rearrange(
                    "p (j k two) -> p j k two", two=2, k=K2)[:, :, :, ::-1]
                m1 = prodp.tile([128, UW], f16, tag=f"m1_{u}",
                                name=f"m1_{u}")
                m2 = prodp.tile([128, UW], f16, tag=f"m2_{u}",
                                name=f"m2_{u}")
                m1v = m1[:, :].rearrange("p (j d) -> p j d", d=D)
                m2v = m2[:, :].rearrange(
                    "p (j k two) -> p j k two", two=2, k=K2)
                nc.vector.tensor_mul(m1v, xv, cb)
                nc.vector.tensor_mul(m2v, xsw, sdb)
                m1s.append(m1)
                m2s.append(m2)
                if u == NU - 2:
                    # combine unit 6 between the stage-6 and stage-7
                    # multiplies so its output drains under the final
                    # muls; its DMA is issued after the PE units' (the
                    # in-order sync DGE must dispatch by readiness)
                    o6 = odvep.tile([128, UW], f16, tag="od", name="o6")
                    nc.vector.tensor_add(o6[:, :], m1[:, :], m2[:, :])

            for u in range(NU):
                if u == NU - 2:
                    nc.sync.dma_start(OUT[u], o6[:, :])
                    continue
                o = outp.tile([128, UW], f16, tag="o", name=f"o{u}")
                if u in PE_UNITS:
                    ps = psp.tile([128, UW], f32, tag="ps", name=f"ps{u}")
                    for q in range(UW // 512):
                        qs = slice(512 * q, 512 * (q + 1))
                        nc.tensor.matmul(ps[:, qs], idt, m1s[u][:, qs],
                                         start=True, stop=False)
                        nc.tensor.matmul(ps[:, qs], idt, m2s[u][:, qs],
                                         start=False, stop=True)
                    nc.scalar.copy(o[:, :], ps[:, :])
                    nc.sync.dma_start(OUT[u], o[:, :])
                else:
                    # final unit in shrinking pieces for a short drain
                    for a, z in ((0, 1024), (1024, 1536), (1536, 2048)):
                        nc.vector.tensor_add(o[:, a:z], m1s[u][:, a:z],
                                             m2s[u][:, a:z])
                        nc.sync.dma_start(OUT[u][:, a:z], o[:, a:z])
    nc.compile()
    return nc


def _get_built():
    if "nc" not in _cache:
        _cache["nc"] = _build_nc()
    return _cache["nc"]


def kernel(x, W, b):
    from concourse.bass_utils import run_bass_kernel_spmd

    assert x.shape == (B, S, H, D)
    x6 = np.asarray(x, dtype=np.float32).reshape(
        B, NCORES, NSB, 128, H, D).astype(np.float16)
    xs = np.ascontiguousarray(x6.transpose(1, 2, 0, 3, 4, 5)).reshape(
        NCORES, NU, 128, UW)

    if "tabs" not in _cache:
        CDf, SDf = _make_tables()
        cc = CDf.reshape(NCORES, NSB, 128, D)
        ss = SDf.reshape(NCORES, NSB, 128, D)
        tabs = np.empty((NCORES, 128, NSB * 2 * D + 128), np.float16)
        for sb in range(NSB):
            tabs[:, :, sb * 2 * D:sb * 2 * D + D] = cc[:, sb]
            tabs[:, :, sb * 2 * D + D:sb * 2 * D + 2 * D] = ss[:, sb]
        tabs[:, :, NSB * 2 * D:] = np.eye(128, dtype=np.float16)[None]
        _cache["tabs"] = np.ascontiguousarray(tabs)
    tabs = _cache["tabs"]

    nc = _get_built()
    in_maps = []
    for c in range(NCORES):
        in_maps.append({"X": xs[c], "TAB": tabs[c]})
    res = run_bass_kernel_spmd(nc, in_maps, core_ids=list(range(NCORES)))
    if res.exec_time_ns is not None:
        print(f"HW exec time: {res.exec_time_ns} ns")

    outs = np.stack([res.results[c]["OUT"] for c in range(NCORES)])
    full = outs.reshape(NCORES, NSB, B, 128, H, D).transpose(2, 0, 1, 3, 4, 5)
    return np.ascontiguousarray(full.reshape(B, S, H, D).astype(np.float32))
